# revision 1
# baseline (speedup 1.0000x reference)
"""Fused single-head attention kernel for 8 TRN2 NeuronCores.

Problem: B=4, S=2048, D=1024 attention:
    Q = x @ Wq.T + bq; K = x @ Wk.T + bk; V = x @ Wv.T + bv
    out = softmax(Q K^T / sqrt(D)) @ V

Sharding (no cross-core traffic): core c handles batch b = c//2 and
query half h = c%2 (1024 queries).

The kernel uses an algebraic refactoring that removes the K and V
projections (and with them any need to exchange K/V between the two
cores of a batch pair):

  logits = (x_q Wq^T + bq)(x_k Wk^T + bk)^T / sqrt(D)
         = x_q M2 x_k^T + x_k.z + (per-q terms), M2 = Wq^T Wk / sqrt(D)
  - the per-QUERY additive terms (x_q Wq^T bk and bq.bk) are constant
    along the softmax axis and drop out of the softmax entirely;
  - the per-KEY term x_k.(Wk^T bq)/sqrt(D) = x_k.z survives and is
    folded into Y (Y' = M2^T x_q^T + z 1^T, added as the per-partition
    bias of Y's evacuation), so it costs nothing.
  M2 [d,d] and z [d] depend only on the weights and are precomputed on
  the host (f64) - weight fusion, no runtime data involved.

  out = softmax @ (x Wv^T) + bv = ((P x) Wv^T)/rowsum(P) + bv
  so V is never materialized: first tmp = P^T.T @ x (attention-weighted
  inputs), then one [1024,1024] projection by Wv^T at the end.

Per-core device FLOPs drop from 15.0 to 12.9 GFLOP and all tensors are
core-local. x is passed in two host-prepared layouts, xT [d,s] and
xN [s,d], both rotated along s by -h*QH so this core's queries are
always positions 0:QH (a uniform slice; attention is permutation-
invariant along the key axis, and xT/xN agree on the rotation).

Device dataflow (all matmul inputs bf16, accumulation fp32), tuned so
the PE stream never waits on another engine (the PE is the binding
resource: ~770 matmuls x (512 cols + ~76 cycles of ldweights/issue
overhead each) ~ 189us at 2.4 GHz; the softmax/normalization work is
kept entirely off the PE):
  Y[dc,q] = M2^T.T @ xqT + z   (sb-outer/dc-inner so pass1's first
    group only depends on an evac finished 8 groups earlier)
  per q-block of 512 (pass1/2/3):
    pass1: attT[k,q] += xT_slice^T.T @ Y  (k on partitions);
      PT = exp(attT) (ScalarE); the DVE accumulates racc += PT per
      k-tile (softmax denominators, no PE involvement)
    after pass1: GpSimd partition_all_reduce makes every partition of
      srow_all hold rowsum[0:512]; DVE 32x32 block transposes put
      rowsum[qs*128+p] on partition p, DVE reciprocal -> 1/S [128,1]
      per q-subtile.  Zero PE instructions for the whole softmax
      normalization (the old version burned 16 ones-matmuls + 4
      transpose matmuls + a ScalarE PSUM copy on the PE path).
    pass2: tmpT[d,q] += xN_slice^T.T @ PT  (PSUM over 16 k-tiles)
    pass3: out[q,e] += tmpT_slice^T.T @ WvT; one DVE
      scalar_tensor_tensor evac: out = pso * (1/S) + bv, stored bf16
      (host upcasts to f32), stores alternating between the sync and
      gpsimd HWDGE queues.
  Weight streams (M2, WvT reloaded per rep) ride the scalar HWDGE
  queue, which is idle in steady state, so they prefetch during the
  previous rep instead of queueing behind the output stores on the
  sync queue.  The prologue stages m2 columns 0:128 and the query-half
  of xT first, so the first Y matmul starts ~2.4us after launch.
  PSUM: psA=4 banks (Y/pass1/pass3) + psT=3 (pass2); measured faster
  than 3+3, 5+2, and any variant with the row-sum on PSUM or Pool.
"""

import os
import sys

for _p in ("/opt/trn_rl_repo", "/root/.axon_site/_ro/trn_rl_repo"):
    if os.path.isdir(_p) and _p not in sys.path:
        sys.path.insert(0, _p)

import numpy as np
import ml_dtypes

import concourse.bass as bass
import concourse.tile as tile
from concourse import bacc, bass_isa, mybir
from concourse.bass_utils import run_bass_kernel_spmd

BF16 = ml_dtypes.bfloat16
F32 = mybir.dt.float32
CDT = mybir.dt.bfloat16

B, S, D = 4, 2048, 1024
N_CORES = 8
P = 128
DT = D // P          # 8 d-tiles (contraction)
KT_N = S // P        # 16 k-tiles
QH = S // 2          # 1024 queries per core
QB = 512             # q-block for phase B
NQB = QH // QB       # 2 q-blocks
QS = QB // P         # 4 q-subtiles per block

_NC_CACHE = {}


def build_nc(reps: int = 1, mode: str = "full"):
    nc = bacc.Bacc("TRN2", target_bir_lowering=False, debug=False,
                   num_devices=N_CORES)
    Exp = mybir.ActivationFunctionType.Exp
    Copy = mybir.ActivationFunctionType.Copy

    xT_d = nc.dram_tensor("xT", [D, S], CDT, kind="ExternalInput").ap()
    xN_d = nc.dram_tensor("xN", [S, D], CDT, kind="ExternalInput").ap()
    m2_d = nc.dram_tensor("M2", [D, D], CDT, kind="ExternalInput").ap()
    wvT_d = nc.dram_tensor("WvT", [D, D], CDT, kind="ExternalInput").ap()
    z_d = nc.dram_tensor("z2", [P, DT], F32, kind="ExternalInput").ap()
    bv_d = nc.dram_tensor("bvb", [P, D], F32, kind="ExternalInput").ap()
    out_d = nc.dram_tensor("out", [QH, D], CDT, kind="ExternalOutput").ap()

    with tile.TileContext(nc) as tc:
        with (
            tc.tile_pool(name="resident", bufs=1) as res,
            tc.tile_pool(name="wpool", bufs=2) as wpool,
            tc.tile_pool(name="pt", bufs=2) as ptpool,
            tc.tile_pool(name="tm", bufs=2) as tmpool,
            tc.tile_pool(name="racc", bufs=2) as rpool,
            tc.tile_pool(name="osb", bufs=4) as opool,
            tc.tile_pool(name="small", bufs=4) as spool,
            tc.tile_pool(name="ps", bufs=4, space="PSUM") as psA,
            tc.tile_pool(name="ptm", bufs=3, space="PSUM") as psT,
        ):
            # ---- resident loads (once) ----
            # the Y matmuls need z + M2 + xqT first; xT (pass1) next;
            # xN (pass2) and bv (epilogue) last. xT/xN ride both HWDGE
            # queues so they stream in parallel.
            z_sb = res.tile([P, DT], F32, tag="z", name="z_sb")
            nc.scalar.dma_start(z_sb[:], z_d[:, :])
            # staged m2: the first Y group only touches columns 0:128
            # of every m2 tile, so land those first (shaves most of the
            # cold-start wait before the first matmul)
            m2 = [wpool.tile([P, D], CDT, tag=f"w{d}", name=f"m2_{d}")
                  for d in range(DT)]
            for d in range(DT):
                nc.sync.dma_start(m2[d][:, 0:P], m2_d[d * P:(d + 1) * P, 0:P])
            # xt halves: Y's rhs only needs columns 0:QH, so land those
            # first (2KB/partition descriptors either way)
            # prologue loads fan out over the three DMA-capable queues
            # (SP / Activation / GpSimd) so a cold single-shot start is
            # not gated by the bandwidth of one queue
            ldq = [nc.sync, nc.scalar, nc.gpsimd]
            xt = [res.tile([P, S], CDT, tag=f"xt{d}", name=f"xt{d}")
                  for d in range(DT)]
            for d in range(DT):
                ldq[d % 3].dma_start(
                    xt[d][:, 0:QH], xT_d[d * P:(d + 1) * P, 0:QH])
            for d in range(DT):
                nc.sync.dma_start(m2[d][:, P:D], m2_d[d * P:(d + 1) * P, P:D])
            for d in range(DT):
                ldq[d % 3].dma_start(
                    xt[d][:, QH:S], xT_d[d * P:(d + 1) * P, QH:S])
            xn = [res.tile([P, D], CDT, tag=f"xn{k}", name=f"xn{k}")
                  for k in range(KT_N)]
            for k in range(KT_N):
                ldq[k % 3].dma_start(
                    xn[k][:], xN_d[k * P:(k + 1) * P, :])
            bv_sb = res.tile([P, D], F32, tag="bv", name="bv_sb")
            nc.scalar.dma_start(bv_sb[:], bv_d[:, :])

            yt = [res.tile([P, QH], CDT, tag=f"yt{d}", name=f"yt{d}")
                  for d in range(DT)]

            if mode in ("MM", "MMD", "MME"):
                # PE-only diagnostic: the same matmul stream with no
                # cross-engine consumers (PSUM recycled WAW-only).
                if mode != "MME":
                    ptres = [res.tile([P, QB], CDT, tag=f"pr{k}",
                                      name=f"pr{k}") for k in range(KT_N)]
                    tmres = [res.tile([P, 512], CDT, tag=f"tr{d}",
                                      name=f"tr{d}") for d in range(DT)]
                    for d in range(DT):
                        nc.vector.memset(tmres[d][:], 1.25)
                    for k in range(KT_N):
                        nc.vector.memset(ptres[k][:], 0.93)
                wvr = [res.tile([P, D], CDT, tag=f"wvr{d}",
                                name=f"wvr{d}") for d in range(DT)]
                for d in range(DT):
                    nc.vector.memset(yt[d][:], 0.0005)
                    nc.vector.memset(wvr[d][:], -0.73)
                if mode == "MMD":
                    dumb = [res.tile([P, 512], CDT, tag=f"dum{i}",
                                     name=f"dum{i}") for i in range(2)]
                    for i in range(2):
                        nc.vector.memset(dumb[i][:], 0.1)
                for _i_rep in range(reps):
                    if mode == "MMD" and _i_rep > 0:
                        m2 = [wpool.tile([P, D], CDT, tag=f"w{d}",
                                         name=f"m2_{d}")
                              for d in range(DT)]
                        for d in range(DT):
                            nc.scalar.dma_start(
                                m2[d][:], m2_d[d * P:(d + 1) * P, :])
                    for sb in range(QH // 512):
                        for dc in range(DT):
                            ps = psA.tile([P, 512], F32, tag="ps",
                                          name="ps")
                            for d in range(DT):
                                nc.tensor.matmul(
                                    ps[:],
                                    lhsT=m2[d][:, dc * P:(dc + 1) * P],
                                    rhs=xt[d][:, sb * 512:(sb + 1) * 512],
                                    start=(d == 0), stop=(d == DT - 1))
                            if mode == "MME":
                                nc.scalar.activation(
                                    yt[dc][:, sb * 512:(sb + 1) * 512],
                                    ps[:],
                                    mybir.ActivationFunctionType.Identity,
                                    bias=z_sb[:, dc:dc + 1])
                    if mode == "MMD":
                        wvd = [wpool.tile([P, D], CDT, tag=f"w{d}",
                                          name=f"wv_{d}")
                               for d in range(DT)]
                        for d in range(DT):
                            nc.scalar.dma_start(
                                wvd[d][:], wvT_d[d * P:(d + 1) * P, :])
                    for qb in range(NQB):
                        racc = (rpool.tile([P, QB], F32, tag="racc",
                                           name="racc")
                                if mode == "MME" else None)
                        pts = []
                        for k in range(KT_N):
                            psa = psA.tile([P, QB], F32, tag="ps",
                                           name="psa")
                            for d in range(DT):
                                nc.tensor.matmul(
                                    psa[:],
                                    lhsT=xt[d][:, k * P:(k + 1) * P],
                                    rhs=yt[d][:, qb * QB:(qb + 1) * QB],
                                    start=(d == 0), stop=(d == DT - 1))
                            if mode == "MME":
                                pt_sb = ptpool.tile(
                                    [P, QB], CDT, tag=f"pt{k}",
                                    name=f"pt_sb{k}")
                                nc.scalar.activation(
                                    pt_sb[:], psa[:],
                                    mybir.ActivationFunctionType.Exp)
                                if k == 0:
                                    nc.vector.tensor_copy(
                                        out=racc[:], in_=pt_sb[:])
                                else:
                                    nc.vector.tensor_add(
                                        racc[:], racc[:], pt_sb[:])
                                pts.append(pt_sb)
                        tms = []
                        for dt_i in range(DT):
                            pst = psT.tile([P, 512], F32, tag="ptm",
                                           name="pst")
                            for k in range(KT_N):
                                nc.tensor.matmul(
                                    pst[:],
                                    lhsT=xn[k][:,
                                               dt_i * P:(dt_i + 1) * P],
                                    rhs=(pts[k] if mode == "MME"
                                         else ptres[k])[:],
                                    start=(k == 0), stop=(k == KT_N - 1))
                            if mode == "MME":
                                tm = tmpool.tile([P, 512], CDT,
                                                 tag=f"tm{dt_i}",
                                                 name=f"tm{dt_i}")
                                nc.vector.tensor_copy(out=tm[:],
                                                      in_=pst[:])
                                tms.append(tm)
                        for qs in range(QS):
                            for eb in range(2):
                                pso = psA.tile([P, 512], F32, tag="ps",
                                               name="pso")
                                for dt_i in range(DT):
                                    lhs_t = (tms[dt_i]
                                             if mode == "MME"
                                             else tmres[dt_i])
                                    nc.tensor.matmul(
                                        pso[:],
                                        lhsT=lhs_t[:,
                                                   qs * P:(qs + 1) * P],
                                        rhs=wvr[dt_i][:,
                                                      eb * 512:(eb + 1) * 512],
                                        start=(dt_i == 0),
                                        stop=(dt_i == DT - 1))
                                if mode == "MME":
                                    osb = opool.tile([P, 512], F32,
                                                     tag="osb",
                                                     name="osb")
                                    nc.scalar.activation(
                                        osb[:], pso[:],
                                        mybir.ActivationFunctionType.Copy)
                                    nc.vector.tensor_add(
                                        osb[:], osb[:],
                                        bv_sb[:, eb * 512:(eb + 1) * 512])
                                elif mode == "MMD":
                                    row = qb * QB + qs * P
                                    nc.sync.dma_start(
                                        out_d[row:row + P,
                                              eb * 512:(eb + 1) * 512],
                                        dumb[eb][:])
                psl = psA.tile([P, 8], F32, tag="ps", name="psl")
                nc.tensor.matmul(psl[:], lhsT=m2[0][:, 0:P],
                                 rhs=xt[0][:, 0:8], start=True, stop=True)
                osl = opool.tile([P, 8], CDT, tag="osb", name="osl")
                nc.vector.tensor_copy(out=osl[:], in_=psl[:])
                nc.sync.dma_start(out_d[0:P, 0:8], osl[:])
                a_iters = []
                b_iters = []
            else:
                a_iters = (range(reps) if mode in ("full", "A")
                           else range(1))
                b_iters = (range(reps)
                           if mode in ("full", "B", "B1", "B2",
                                       "B3", "B4")
                           else range(1))
            wv = None
            for _i_rep, _rep in enumerate(a_iters):
                # ---- Y = M2^T.T @ xqT : [dc, q] ----
                if _i_rep > 0:
                    m2 = [wpool.tile([P, D], CDT, tag=f"w{d}",
                                     name=f"m2_{d}") for d in range(DT)]
                    for d in range(DT):
                        nc.scalar.dma_start(m2[d][:],
                                            m2_d[d * P:(d + 1) * P, :])
                # sb outer / dc inner: pass1's first group (needs all
                # yt[*][:, 0:512]) depends only on the sb=0 evacs, all
                # finished while the sb=1 groups stream.
                for sb in range(QH // 512):
                    for dc in range(DT):
                        ps = psA.tile([P, 512], F32, tag="ps", name="ps")
                        for d in range(DT):
                            nc.tensor.matmul(
                                ps[:],
                                lhsT=m2[d][:, dc * P:(dc + 1) * P],
                                rhs=xt[d][:, sb * 512:(sb + 1) * 512],
                                start=(d == 0), stop=(d == DT - 1))
                        nc.scalar.activation(
                            yt[dc][:, sb * 512:(sb + 1) * 512], ps[:],
                            mybir.ActivationFunctionType.Identity,
                            bias=z_sb[:, dc:dc + 1])
                # WvT loads reuse the w{d} slots once M2 is consumed;
                # scalar queue = idle, so they prefetch behind m2.
                wv = [wpool.tile([P, D], CDT, tag=f"w{d}", name=f"wv_{d}")
                      for d in range(DT)]
                for d in range(DT):
                    nc.scalar.dma_start(wv[d][:],
                                        wvT_d[d * P:(d + 1) * P, :])

            # diagnostic strip levels for the body ("B" = everything):
            # B1 drops racc/srow/scol (constant 1/S), B2 also drops the
            # pass-3 evac + store, B3 also reads resident tm, B4 also
            # drops exp (pass 2 reads resident pt) -> pure matmul body.
            strip = int(mode[1]) if mode in ("B1", "B2", "B3", "B4") else 0
            do_racc, do_evac, do_tm, do_exp = (
                strip < 1, strip < 2, strip < 3, strip < 4)
            if not (do_tm and do_exp):
                ptres2 = [res.tile([P, QB], CDT, tag=f"p2r{k}",
                                   name=f"p2r{k}") for k in range(KT_N)]
                tmres2 = [res.tile([P, 512], CDT, tag=f"t2r{d}",
                                   name=f"t2r{d}") for d in range(DT)]
                for d in range(DT):
                    nc.vector.memset(tmres2[d][:], 1.25)
                for k in range(KT_N):
                    nc.vector.memset(ptres2[k][:], 0.93)
            for _rep in b_iters:
                for qb in range(NQB):
                    # ---- pass 1: scores, exp, DVE row-sum accum ----
                    # racc lives in PSUM: the DVE accumulation then only
                    # reads pt from SBUF (128KB/k-tile) instead of also
                    # round-tripping a fp32 accumulator through SBUF,
                    # which contends with the PE's stationary+moving
                    # reads during pass 1.
                    # racc accumulated on the (otherwise idle) Pool
                    # engine; the partition reduction + per-partition
                    # 1/S extraction all stay off the PE stream.
                    racc = (rpool.tile([P, QB], F32, tag="racc",
                                       name="racc") if do_racc else None)
                    pts = []
                    for k in range(KT_N):
                        psa = psA.tile([P, QB], F32, tag="ps", name="psa")
                        for d in range(DT):
                            nc.tensor.matmul(
                                psa[:],
                                lhsT=xt[d][:, k * P:(k + 1) * P],
                                rhs=yt[d][:, qb * QB:(qb + 1) * QB],
                                start=(d == 0), stop=(d == DT - 1))
                        if not do_exp:
                            continue
                        pt_sb = ptpool.tile([P, QB], CDT, tag=f"pt{k}",
                                            name=f"pt_sb{k}")
                        nc.scalar.activation(pt_sb[:], psa[:], Exp)
                        if do_racc:
                            # DVE, not Pool: measured ~20us faster here
                            if k == 0:
                                nc.vector.tensor_copy(out=racc[:],
                                                      in_=pt_sb[:])
                            else:
                                nc.vector.tensor_add(racc[:], racc[:],
                                                     pt_sb[:])
                        pts.append(pt_sb)
                    recs = []
                    if do_racc:
                        # all-reduce across partitions: every partition
                        # of srow_all holds the full rowsum[0:512]
                        srow_all = rpool.tile([P, QB], F32, tag="srall",
                                              name="srow_all")
                        nc.gpsimd.partition_all_reduce(
                            srow_all[:], racc[:], channels=P,
                            reduce_op=bass_isa.ReduceOp.add)
                        # 32x32 block transposes put rowsum[qs*128+p]
                        # on partition p; reciprocal -> per-q 1/S
                        for qs in range(QS):
                            st = spool.tile([P, 32], F32, tag="st",
                                            name=f"st{qs}")
                            for i in range(4):
                                lo, hi = 32 * i, 32 * (i + 1)
                                nc.vector.transpose(
                                    st[lo:hi, :],
                                    srow_all[lo:hi,
                                             qs * P + lo:qs * P + hi])
                            rec = spool.tile([P, 1], F32, tag="rec",
                                             name="rec")
                            nc.vector.reciprocal(rec[:], st[:, 0:1])
                            recs.append(rec)
                    # ---- pass 2: tmpT[d, q] = sum_k x_k^T P^T ----
                    tms = []
                    for dt_i in range(DT):
                        pst = psT.tile([P, 512], F32, tag="ptm",
                                       name="pst")
                        for k in range(KT_N):
                            nc.tensor.matmul(
                                pst[:],
                                lhsT=xn[k][:, dt_i * P:(dt_i + 1) * P],
                                rhs=(pts[k] if do_exp
                                     else ptres2[k])[:],
                                start=(k == 0), stop=(k == KT_N - 1))
                        if do_tm:
                            tm = tmpool.tile([P, 512], CDT,
                                             tag=f"tm{dt_i}",
                                             name=f"tm{dt_i}")
                            nc.vector.tensor_copy(out=tm[:], in_=pst[:])
                            tms.append(tm)
                    # ---- pass 3: out[q, e] = tmpT^T @ WvT, *1/S + bv --
                    for qs in range(QS):
                        for eb in range(2):
                            pso = psA.tile([P, 512], F32, tag="ps",
                                           name="pso")
                            for dt_i in range(DT):
                                lhs_p3 = (tms[dt_i] if do_tm
                                          else tmres2[dt_i])
                                nc.tensor.matmul(
                                    pso[:],
                                    lhsT=lhs_p3[:, qs * P:(qs + 1) * P],
                                    rhs=wv[dt_i][:, eb * 512:(eb + 1) * 512],
                                    start=(dt_i == 0),
                                    stop=(dt_i == DT - 1))
                            if not do_evac:
                                continue
                            # bf16 store (host upcasts): halves store
                            # bytes; stores alternate between the sync
                            # and gpsimd HWDGE queues so the drain never
                            # backs up into osb reuse.
                            osb = opool.tile([P, 512], CDT, tag="osb",
                                             name="osb")
                            # one DVE op: out = pso * (1/S) + bv
                            nc.vector.scalar_tensor_tensor(
                                osb[:], pso[:],
                                (recs[qs][:] if do_racc
                                 else 1.0 / 2048.0),
                                bv_sb[:, eb * 512:(eb + 1) * 512],
                                op0=mybir.AluOpType.mult,
                                op1=mybir.AluOpType.add)
                            row = qb * QB + qs * P
                            (nc.sync if eb == 0 else nc.gpsimd).dma_start(
                                out_d[row:row + P, eb * 512:(eb + 1) * 512],
                                osb[:])
            if mode == "A":
                nc.gpsimd.dma_start(out_d[0:P, 0:8], yt[0][:, 0:8])
            elif mode in ("B2", "B3", "B4"):
                nc.gpsimd.dma_start(out_d[0:P, 0:8], yt[0][:, 0:8])
    nc.compile()
    return nc


def _get_nc(reps: int = 1, mode: str = "full"):
    key = (reps, mode)
    if key not in _NC_CACHE:
        _NC_CACHE[key] = build_nc(reps, mode)
    return _NC_CACHE[key]


def make_in_maps(x, Wq, bq, Wk, bk, Wv, bv):
    inv = np.float64(1.0 / np.sqrt(D))
    M2 = Wq.T.astype(np.float64) @ Wk.astype(np.float64) * inv
    z = Wk.T.astype(np.float64) @ bq.astype(np.float64) * inv
    m2b = np.ascontiguousarray(M2.astype(np.float32)).astype(BF16)
    wvT = np.ascontiguousarray(Wv.T).astype(BF16)
    z2 = np.ascontiguousarray(
        z.astype(np.float32).reshape(DT, P).T).astype(np.float32)
    bvb = np.ascontiguousarray(np.broadcast_to(bv, (P, D))).astype(np.float32)
    in_maps = []
    for c in range(N_CORES):
        b, h = divmod(c, 2)
        # rotate the sequence axis so this core's query half is always
        # columns/rows 0:QH -- attention is permutation-invariant along
        # the key axis as long as xT (pass1) and xN (pass2) agree.
        xr = np.roll(x[b], -h * QH, axis=0)
        xT = np.ascontiguousarray(xr.T).astype(BF16)
        xN = np.ascontiguousarray(xr).astype(BF16)
        in_maps.append({
            "xT": xT, "xN": xN,
            "M2": m2b, "WvT": wvT,
            "z2": z2, "bvb": bvb,
        })
    return in_maps


def kernel(x, Wq, bq, Wk, bk, Wv, bv):
    x = np.asarray(x, np.float32)
    in_maps = make_in_maps(x, np.asarray(Wq, np.float32),
                           np.asarray(bq, np.float32),
                           np.asarray(Wk, np.float32),
                           np.asarray(bk, np.float32),
                           np.asarray(Wv, np.float32),
                           np.asarray(bv, np.float32))
    nc = _get_nc()
    res = run_bass_kernel_spmd(nc, in_maps, core_ids=list(range(N_CORES)))
    out = np.empty((B, S, D), np.float32)
    for c in range(N_CORES):
        b, h = divmod(c, 2)
        out[b, h * QH:(h + 1) * QH, :] = np.asarray(
            res.results[c]["out"], dtype=np.float32)
    return out

